# revision 16
# baseline (speedup 1.0000x reference)
"""Locally-connected (masked linear) layer for 8 TRN2 NeuronCores.

y = x @ (W * M)^T + b
  x: [4096, 4096] f32, W/M: [4096, 4096] f32, b: [4096] f32.

Strategy (tensor-parallel over out_features):
  - Each core owns a 512-row shard of the output features.
  - The mask multiply (W * M) is exact in bf16 (M is 0/1), so it is folded
    on the host; the device only sees the pre-masked, pre-transposed weight.
  - Host uploads contraction-major, DMA-friendly layouts (>=4KB contiguous
    per partition per transfer):
      xP[p, g, k, b]  = x[g*512+b, k*128+p]          (bf16, shared by cores)
      wP[p, k, o]     = (W*M)[c*512+o, k*128+p]      (bf16, per core)
      bT[p, j]        = bias[c*512 + j*128 + p]      (f32, per core)
  - Device: per batch group g (512 cols), accumulate y over 32 k-tiles in
    fp32 PSUM (4 PSUM banks = 4 out-feature chunks of 128), add bias on
    evacuation, store y^T shard as bf16.  The weight DMA is chunked so the
    matmul stream starts as soon as the first chunks land (~13us: framework
    preamble + DMA ramp); next group's x is prefetched one group ahead.
    The last group runs j-outer/k-inner so only the final 128-row
    evacuation remains in the tail.
  - Host concatenates the 8 shards, inverts the permutation, casts to f32.
"""

import os

import numpy as np
import ml_dtypes

BATCH = 4096
IN_F = 4096
OUT_F = 4096
N_CORES = 8
O_SHARD = OUT_F // N_CORES  # 512
P = 128                     # SBUF partitions
BG = 512                    # batch columns per PSUM accumulation group
WARMUP = 7                  # scratch matmuls to lift the PE HAM clock gate:
                            # sized so the (cold-rate) warmup chain ends
                            # right as the first x/w chunks land (~11.5us),
                            # so the real stream starts warm with no PE idle


def _chunks(kt):
    """k-tile DMA chunk sizes. Two 2-tile leading chunks cut the packet
    count gating the first matmul (early DMA is latency-limited per
    packet), then 4-tile chunks for the bandwidth phase."""
    if kt <= 4:
        return [2, kt - 2] if kt > 2 else [kt]
    out = [2, 2]
    out += [4] * ((kt - 4) // 4)
    rem = kt - sum(out)
    if rem:
        out.append(rem)
    return out

_BF16 = ml_dtypes.bfloat16
_NC = None
LAST_RESULT = None


def _ensure_axon_hooks_stub():
    """bass_utils' axon trace path imports antenv.axon_hooks, which this
    container's antenv stub lacks. Install a minimal registry so the
    import succeeds (hook None => bass_utils skips tracing gracefully)."""
    import sys
    import types

    try:
        import antenv.axon_hooks  # noqa: F401
        return
    except ImportError:
        pass
    import antenv

    mod = types.ModuleType("antenv.axon_hooks")
    mod._HOOK = None

    def set_axon_ntff_profile_hook(h):
        mod._HOOK = h

    def get_axon_ntff_profile_hook():
        return mod._HOOK

    mod.set_axon_ntff_profile_hook = set_axon_ntff_profile_hook
    mod.get_axon_ntff_profile_hook = get_axon_ntff_profile_hook
    antenv.axon_hooks = mod
    sys.modules["antenv.axon_hooks"] = mod


def _install_real_ntff_hook():
    """Wire the ctypes NTFF profiling hook (normally registered by the
    boot middleware) so run_bass_kernel_spmd(trace=True) works."""
    _ensure_axon_hooks_stub()
    import antenv.axon_hooks as ah

    if ah.get_axon_ntff_profile_hook() is None:
        try:
            from trn_agent_boot.trn_boot import _ntff_profile_via_ctypes

            hook = _ntff_profile_via_ctypes("/opt/axon/libaxon_pjrt.so")
            if hook is not None:
                ah.set_axon_ntff_profile_hook(hook)
        except Exception:
            pass
    try:
        import concourse.bass_utils as bu

        bu.upload_artifacts = lambda tmpdir: "local://" + str(tmpdir)
    except Exception:
        pass


def build_nc(batch=BATCH, in_f=IN_F, o_shard=O_SHARD, bg=BG, warmup=WARMUP):
    import concourse.mybir as mybir
    from concourse import bacc
    from concourse.tile import TileContext

    p = P
    kt = in_f // p          # k tiles along contraction
    oc = o_shard // p       # out-feature chunks of 128
    ng = batch // bg        # batch groups
    chunks = _chunks(kt)    # k-tile chunk schedule for x and w DMA
    bf16 = mybir.dt.bfloat16
    f32 = mybir.dt.float32

    nc = bacc.Bacc()
    # flattened layouts; per-partition runs are contiguous per chunk
    xP = nc.declare_dram_parameter("xP", [p, ng * kt * bg], bf16,
                                   isOutput=False)
    wP = nc.declare_dram_parameter("wP", [p, kt * o_shard], bf16,
                                   isOutput=False)
    bT = nc.declare_dram_parameter("bT", [p, oc], f32, isOutput=False)
    yP = nc.declare_dram_parameter("yP", [p, ng * oc * bg], bf16,
                                   isOutput=True)

    with TileContext(nc) as tc:
        with tc.tile_pool(name="const", bufs=1) as cpool, \
             tc.tile_pool(name="xin", bufs=3) as xpool, \
             tc.tile_pool(name="acc", bufs=8, space="PSUM") as ppool, \
             tc.tile_pool(name="out", bufs=4) as opool:

            # 1-packet dummy DMAs prime the descriptor-generation path of
            # both trigger engines' queues so the real chunk0 packets flow
            # at the pipelined rate instead of paying first-byte latency
            dwu = cpool.tile([1, 32], bf16)
            dxu = cpool.tile([1, 32], bf16)
            nc.sync.dma_start(out=dwu, in_=wP[0:1, 0:32])
            nc.gpsimd.dma_start(out=dxu, in_=xP[0:1, 0:32])

            # scratch matmuls keep the PE busy from the end of the
            # framework preamble so the HAM clock gate opens (1.2 ->
            # 2.4 GHz) before the first real matmul's operands land
            if warmup:
                wu_w = cpool.tile([p, p], bf16)
                wu_x = cpool.tile([p, bg], bf16)
                wu_ps = ppool.tile([p, bg], f32, tag="ps", name="wu")
                nc.vector.memset(wu_w, 0.0)
                nc.vector.memset(wu_x, 0.0)
                for i in range(warmup):
                    nc.tensor.matmul(wu_ps, wu_w, wu_x,
                                     start=(i == 0),
                                     stop=(i == warmup - 1))

            # masked weights resident in SBUF; w chunks trigger on the
            # sync engine while x chunks trigger on gpsimd so the two
            # streams start in parallel (each DMA trigger costs ~0.6us
            # of engine time)
            mw = cpool.tile([p, kt * o_shard], bf16)
            xg0 = xpool.tile([p, kt * bg], bf16, tag="x")
            k0 = 0
            for ch in chunks:
                nc.sync.dma_start(
                    out=mw[:, k0 * o_shard:(k0 + ch) * o_shard],
                    in_=wP[:, k0 * o_shard:(k0 + ch) * o_shard],
                )
                nc.gpsimd.dma_start(
                    out=xg0[:, k0 * bg:(k0 + ch) * bg],
                    in_=xP[:, k0 * bg:(k0 + ch) * bg],
                )
                k0 += ch

            # bias is tiny and first needed ~40us in; trigger it last
            bias_t = cpool.tile([p, oc], f32)
            nc.sync.dma_start(out=bias_t, in_=bT[:])

            xg = xg0
            for g in range(ng):
                # prefetch next group's x one group ahead of use
                if g + 1 < ng:
                    xn = xpool.tile([p, kt * bg], bf16, tag="x")
                    base = (g + 1) * kt * bg
                    k0 = 0
                    for ch in chunks:
                        nc.gpsimd.dma_start(
                            out=xn[:, k0 * bg:(k0 + ch) * bg],
                            in_=xP[:, base + k0 * bg:base + (k0 + ch) * bg],
                        )
                        k0 += ch

                if g < ng - 1:
                    # k-outer / j-inner: x chunks are consumed as they land
                    psums = [ppool.tile([p, bg], f32, tag="ps",
                                        name=f"ps{g}_{j}")
                             for j in range(oc)]
                    for k in range(kt):
                        rhs = xg[:, k * bg:(k + 1) * bg]
                        for j in range(oc):
                            nc.tensor.matmul(
                                psums[j],
                                mw[:, k * o_shard + j * p:
                                   k * o_shard + (j + 1) * p],
                                rhs,
                                start=(k == 0),
                                stop=(k == kt - 1),
                            )
                    for j in range(oc):
                        ot = opool.tile([p, bg], bf16, tag="o")
                        nc.vector.tensor_scalar_add(
                            out=ot, in0=psums[j], scalar1=bias_t[:, j:j + 1]
                        )
                        nc.sync.dma_start(
                            out=yP[:, (g * oc + j) * bg:
                                   (g * oc + j + 1) * bg],
                            in_=ot,
                        )
                else:
                    # last group: j-outer / k-inner so earlier j's evacuate
                    # while later j's still compute (shrinks the tail); the
                    # final j evacuates in halves to overlap the bias-add
                    # with the store
                    for j in range(oc):
                        ps = ppool.tile([p, bg], f32, tag="ps",
                                        name=f"ps{g}_{j}")
                        for k in range(kt):
                            nc.tensor.matmul(
                                ps,
                                mw[:, k * o_shard + j * p:
                                   k * o_shard + (j + 1) * p],
                                xg[:, k * bg:(k + 1) * bg],
                                start=(k == 0),
                                stop=(k == kt - 1),
                            )
                        base = (g * oc + j) * bg
                        if j < oc - 1:
                            ot = opool.tile([p, bg], bf16, tag="o")
                            nc.vector.tensor_scalar_add(
                                out=ot, in0=ps, scalar1=bias_t[:, j:j + 1]
                            )
                            nc.sync.dma_start(
                                out=yP[:, base:base + bg], in_=ot)
                        else:
                            h = bg // 2
                            for half in range(2):
                                oth = opool.tile([p, h], bf16, tag="oh")
                                nc.vector.tensor_scalar_add(
                                    out=oth, in0=ps[:, half * h:
                                                    (half + 1) * h],
                                    scalar1=bias_t[:, j:j + 1],
                                )
                                nc.sync.dma_start(
                                    out=yP[:, base + half * h:
                                           base + (half + 1) * h],
                                    in_=oth,
                                )
                xg = xn if g + 1 < ng else None
    nc.finalize()
    return nc


def _prep_in_maps(x, weight, bias, myFilter):
    kt = IN_F // P
    ng = BATCH // BG
    oc = O_SHARD // P
    # xP[p, g, k, b] = x[g*BG+b, k*128+p]
    xb = np.asarray(x, np.float32).astype(_BF16)
    xPb = np.ascontiguousarray(
        xb.reshape(ng, BG, kt, P).transpose(3, 0, 2, 1)
    ).reshape(P, ng * kt * BG)
    mwf = np.asarray(weight, np.float32) * np.asarray(myFilter, np.float32)
    biasf = np.asarray(bias, np.float32)
    in_maps = []
    for c in range(N_CORES):
        rows = slice(c * O_SHARD, (c + 1) * O_SHARD)
        # wP[p, k, o] = mw[c*512+o, k*128+p]
        wPb = np.ascontiguousarray(
            mwf[rows].astype(_BF16).reshape(O_SHARD, kt, P).transpose(2, 1, 0)
        ).reshape(P, kt * O_SHARD)
        bTb = np.ascontiguousarray(biasf[rows].reshape(oc, P).T)
        in_maps.append({"xP": xPb, "wP": wPb, "bT": bTb})
    return in_maps


def kernel(x, weight, bias, myFilter):
    global _NC, LAST_RESULT
    _ensure_axon_hooks_stub()
    from concourse.bass_utils import run_bass_kernel_spmd

    if _NC is None:
        _NC = build_nc()

    in_maps = _prep_in_maps(x, weight, bias, myFilter)

    kwargs = {}
    if os.environ.get("KERNEL_TRACE") == "1":
        _install_real_ntff_hook()
        kwargs["trace"] = True
        tdir = os.environ.get("KERNEL_TRACE_DIR")
        if tdir:
            kwargs["tmpdir"] = tdir

    res = run_bass_kernel_spmd(_NC, in_maps, list(range(N_CORES)), **kwargs)
    LAST_RESULT = res

    kt = IN_F // P
    ng = BATCH // BG
    oc = O_SHARD // P
    # yP[p, g, j, b] -> y[g*BG+b, c*O_SHARD + j*128 + p]
    shards = []
    for c in range(N_CORES):
        yPb = np.asarray(res.results[c]["yP"]).reshape(P, ng, oc, BG)
        shards.append(yPb.transpose(1, 3, 2, 0).reshape(BATCH, O_SHARD))
    y = np.concatenate(shards, axis=1).astype(np.float32)
    return np.ascontiguousarray(y)


# revision 17
# speedup vs baseline: 1.0066x; 1.0066x over previous
"""Locally-connected (masked linear) layer for 8 TRN2 NeuronCores.

y = x @ (W * M)^T + b
  x: [4096, 4096] f32, W/M: [4096, 4096] f32, b: [4096] f32.

Strategy (tensor-parallel over out_features):
  - Each core owns a 512-row shard of the output features.
  - The mask multiply (W * M) is exact in bf16 (M is 0/1), so it is folded
    on the host; the device only sees the pre-masked, pre-transposed weight.
  - Host uploads contraction-major, DMA-friendly layouts (>=4KB contiguous
    per partition per transfer):
      xP[p, g, k, b]  = x[g*512+b, k*128+p]          (bf16, shared by cores)
      wP[p, k, o]     = (W*M)[c*512+o, k*128+p]      (bf16, per core)
      bT[p, j]        = bias[c*512 + j*128 + p]      (f32, per core)
  - Device: per batch group g (512 cols), accumulate y over 32 k-tiles in
    fp32 PSUM (4 PSUM banks = 4 out-feature chunks of 128), add bias on
    evacuation, store y^T shard as bf16.  The weight DMA is chunked so the
    matmul stream starts as soon as the first chunks land (~13us: framework
    preamble + DMA ramp); next group's x is prefetched one group ahead.
    The last group runs j-outer/k-inner so only the final 128-row
    evacuation remains in the tail.
  - Host concatenates the 8 shards, inverts the permutation, casts to f32.
"""

import os

import numpy as np
import ml_dtypes

BATCH = 4096
IN_F = 4096
OUT_F = 4096
N_CORES = 8
O_SHARD = OUT_F // N_CORES  # 512
P = 128                     # SBUF partitions
BG = 512                    # batch columns per PSUM accumulation group
WARMUP = 13                 # scratch matmuls to lift the PE HAM clock gate:
                            # sized so the (cold-rate) warmup chain ends
                            # right as the first x/w chunks land (~13.3us),
                            # so the real stream starts warm with no PE idle


def _chunks(kt):
    """k-tile DMA chunk sizes. Uniform 4-tile chunks: early DMA is
    latency-limited (~350ns/packet/engine), so the first chunk cannot
    land much before ~13us regardless of size, and smaller chunks starve
    the matmul stream during the bandwidth ramp (measured A/B)."""
    out = [4] * (kt // 4)
    rem = kt - sum(out)
    if rem:
        out.append(rem)
    return out if out else [kt]

_BF16 = ml_dtypes.bfloat16
_NC = None
LAST_RESULT = None


def _ensure_axon_hooks_stub():
    """bass_utils' axon trace path imports antenv.axon_hooks, which this
    container's antenv stub lacks. Install a minimal registry so the
    import succeeds (hook None => bass_utils skips tracing gracefully)."""
    import sys
    import types

    try:
        import antenv.axon_hooks  # noqa: F401
        return
    except ImportError:
        pass
    import antenv

    mod = types.ModuleType("antenv.axon_hooks")
    mod._HOOK = None

    def set_axon_ntff_profile_hook(h):
        mod._HOOK = h

    def get_axon_ntff_profile_hook():
        return mod._HOOK

    mod.set_axon_ntff_profile_hook = set_axon_ntff_profile_hook
    mod.get_axon_ntff_profile_hook = get_axon_ntff_profile_hook
    antenv.axon_hooks = mod
    sys.modules["antenv.axon_hooks"] = mod


def _install_real_ntff_hook():
    """Wire the ctypes NTFF profiling hook (normally registered by the
    boot middleware) so run_bass_kernel_spmd(trace=True) works."""
    _ensure_axon_hooks_stub()
    import antenv.axon_hooks as ah

    if ah.get_axon_ntff_profile_hook() is None:
        try:
            from trn_agent_boot.trn_boot import _ntff_profile_via_ctypes

            hook = _ntff_profile_via_ctypes("/opt/axon/libaxon_pjrt.so")
            if hook is not None:
                ah.set_axon_ntff_profile_hook(hook)
        except Exception:
            pass
    try:
        import concourse.bass_utils as bu

        bu.upload_artifacts = lambda tmpdir: "local://" + str(tmpdir)
    except Exception:
        pass


def build_nc(batch=BATCH, in_f=IN_F, o_shard=O_SHARD, bg=BG, warmup=WARMUP):
    import concourse.mybir as mybir
    from concourse import bacc
    from concourse.tile import TileContext

    p = P
    kt = in_f // p          # k tiles along contraction
    oc = o_shard // p       # out-feature chunks of 128
    ng = batch // bg        # batch groups
    chunks = _chunks(kt)    # k-tile chunk schedule for x and w DMA
    bf16 = mybir.dt.bfloat16
    f32 = mybir.dt.float32

    nc = bacc.Bacc()
    # flattened layouts; per-partition runs are contiguous per chunk
    xP = nc.declare_dram_parameter("xP", [p, ng * kt * bg], bf16,
                                   isOutput=False)
    wP = nc.declare_dram_parameter("wP", [p, kt * o_shard], bf16,
                                   isOutput=False)
    bT = nc.declare_dram_parameter("bT", [p, oc], f32, isOutput=False)
    yP = nc.declare_dram_parameter("yP", [p, ng * oc * bg], bf16,
                                   isOutput=True)

    with TileContext(nc) as tc:
        with tc.tile_pool(name="const", bufs=1) as cpool, \
             tc.tile_pool(name="xin", bufs=3) as xpool, \
             tc.tile_pool(name="acc", bufs=8, space="PSUM") as ppool, \
             tc.tile_pool(name="out", bufs=4) as opool:

            # scratch matmuls keep the PE busy from the end of the
            # framework preamble so the HAM clock gate opens (1.2 ->
            # 2.4 GHz) before the first real matmul's operands land
            if warmup:
                wu_w = cpool.tile([p, p], bf16)
                wu_x = cpool.tile([p, bg], bf16)
                wu_ps = ppool.tile([p, bg], f32, tag="ps", name="wu")
                nc.vector.memset(wu_w, 0.0)
                nc.vector.memset(wu_x, 0.0)
                for i in range(warmup):
                    nc.tensor.matmul(wu_ps, wu_w, wu_x,
                                     start=(i == 0),
                                     stop=(i == warmup - 1))

            # masked weights resident in SBUF; w chunks trigger on the
            # sync engine while x chunks trigger on gpsimd so the two
            # streams start in parallel (each DMA trigger costs ~0.6us
            # of engine time)
            mw = cpool.tile([p, kt * o_shard], bf16)
            xg0 = xpool.tile([p, kt * bg], bf16, tag="x")
            k0 = 0
            for ch in chunks:
                nc.sync.dma_start(
                    out=mw[:, k0 * o_shard:(k0 + ch) * o_shard],
                    in_=wP[:, k0 * o_shard:(k0 + ch) * o_shard],
                )
                nc.gpsimd.dma_start(
                    out=xg0[:, k0 * bg:(k0 + ch) * bg],
                    in_=xP[:, k0 * bg:(k0 + ch) * bg],
                )
                k0 += ch

            # bias is tiny and first needed ~40us in; trigger it last
            bias_t = cpool.tile([p, oc], f32)
            nc.sync.dma_start(out=bias_t, in_=bT[:])

            xg = xg0
            for g in range(ng):
                # prefetch next group's x one group ahead of use
                if g + 1 < ng:
                    xn = xpool.tile([p, kt * bg], bf16, tag="x")
                    base = (g + 1) * kt * bg
                    k0 = 0
                    for ch in chunks:
                        nc.gpsimd.dma_start(
                            out=xn[:, k0 * bg:(k0 + ch) * bg],
                            in_=xP[:, base + k0 * bg:base + (k0 + ch) * bg],
                        )
                        k0 += ch

                if g < ng - 1:
                    # k-outer / j-inner: x chunks are consumed as they land
                    psums = [ppool.tile([p, bg], f32, tag="ps",
                                        name=f"ps{g}_{j}")
                             for j in range(oc)]
                    for k in range(kt):
                        rhs = xg[:, k * bg:(k + 1) * bg]
                        for j in range(oc):
                            nc.tensor.matmul(
                                psums[j],
                                mw[:, k * o_shard + j * p:
                                   k * o_shard + (j + 1) * p],
                                rhs,
                                start=(k == 0),
                                stop=(k == kt - 1),
                            )
                    for j in range(oc):
                        ot = opool.tile([p, bg], bf16, tag="o")
                        nc.vector.tensor_scalar_add(
                            out=ot, in0=psums[j], scalar1=bias_t[:, j:j + 1]
                        )
                        nc.sync.dma_start(
                            out=yP[:, (g * oc + j) * bg:
                                   (g * oc + j + 1) * bg],
                            in_=ot,
                        )
                else:
                    # last group: j-outer / k-inner so earlier j's evacuate
                    # while later j's still compute (shrinks the tail); the
                    # final j evacuates in halves to overlap the bias-add
                    # with the store
                    for j in range(oc):
                        ps = ppool.tile([p, bg], f32, tag="ps",
                                        name=f"ps{g}_{j}")
                        for k in range(kt):
                            nc.tensor.matmul(
                                ps,
                                mw[:, k * o_shard + j * p:
                                   k * o_shard + (j + 1) * p],
                                xg[:, k * bg:(k + 1) * bg],
                                start=(k == 0),
                                stop=(k == kt - 1),
                            )
                        base = (g * oc + j) * bg
                        if j < oc - 1:
                            ot = opool.tile([p, bg], bf16, tag="o")
                            nc.vector.tensor_scalar_add(
                                out=ot, in0=ps, scalar1=bias_t[:, j:j + 1]
                            )
                            nc.sync.dma_start(
                                out=yP[:, base:base + bg], in_=ot)
                        else:
                            h = bg // 2
                            for half in range(2):
                                oth = opool.tile([p, h], bf16, tag="oh")
                                nc.vector.tensor_scalar_add(
                                    out=oth, in0=ps[:, half * h:
                                                    (half + 1) * h],
                                    scalar1=bias_t[:, j:j + 1],
                                )
                                nc.sync.dma_start(
                                    out=yP[:, base + half * h:
                                           base + (half + 1) * h],
                                    in_=oth,
                                )
                xg = xn if g + 1 < ng else None
    nc.finalize()
    return nc


def _prep_in_maps(x, weight, bias, myFilter):
    kt = IN_F // P
    ng = BATCH // BG
    oc = O_SHARD // P
    # xP[p, g, k, b] = x[g*BG+b, k*128+p]
    xb = np.asarray(x, np.float32).astype(_BF16)
    xPb = np.ascontiguousarray(
        xb.reshape(ng, BG, kt, P).transpose(3, 0, 2, 1)
    ).reshape(P, ng * kt * BG)
    mwf = np.asarray(weight, np.float32) * np.asarray(myFilter, np.float32)
    biasf = np.asarray(bias, np.float32)
    in_maps = []
    for c in range(N_CORES):
        rows = slice(c * O_SHARD, (c + 1) * O_SHARD)
        # wP[p, k, o] = mw[c*512+o, k*128+p]
        wPb = np.ascontiguousarray(
            mwf[rows].astype(_BF16).reshape(O_SHARD, kt, P).transpose(2, 1, 0)
        ).reshape(P, kt * O_SHARD)
        bTb = np.ascontiguousarray(biasf[rows].reshape(oc, P).T)
        in_maps.append({"xP": xPb, "wP": wPb, "bT": bTb})
    return in_maps


def kernel(x, weight, bias, myFilter):
    global _NC, LAST_RESULT
    _ensure_axon_hooks_stub()
    from concourse.bass_utils import run_bass_kernel_spmd

    if _NC is None:
        _NC = build_nc()

    in_maps = _prep_in_maps(x, weight, bias, myFilter)

    kwargs = {}
    if os.environ.get("KERNEL_TRACE") == "1":
        _install_real_ntff_hook()
        kwargs["trace"] = True
        tdir = os.environ.get("KERNEL_TRACE_DIR")
        if tdir:
            kwargs["tmpdir"] = tdir

    res = run_bass_kernel_spmd(_NC, in_maps, list(range(N_CORES)), **kwargs)
    LAST_RESULT = res

    kt = IN_F // P
    ng = BATCH // BG
    oc = O_SHARD // P
    # yP[p, g, j, b] -> y[g*BG+b, c*O_SHARD + j*128 + p]
    shards = []
    for c in range(N_CORES):
        yPb = np.asarray(res.results[c]["yP"]).reshape(P, ng, oc, BG)
        shards.append(yPb.transpose(1, 3, 2, 0).reshape(BATCH, O_SHARD))
    y = np.concatenate(shards, axis=1).astype(np.float32)
    return np.ascontiguousarray(y)


# revision 18
# speedup vs baseline: 1.0074x; 1.0008x over previous
"""Locally-connected (masked linear) layer for 8 TRN2 NeuronCores.

y = x @ (W * M)^T + b
  x: [4096, 4096] f32, W/M: [4096, 4096] f32, b: [4096] f32.

Strategy (tensor-parallel over out_features):
  - Each core owns a 512-row shard of the output features.
  - The mask multiply (W * M) is exact in bf16 (M is 0/1), so it is folded
    on the host; the device only sees the pre-masked, pre-transposed weight.
  - Host uploads contraction-major, DMA-friendly layouts (>=4KB contiguous
    per partition per transfer):
      xP[p, g, k, b]  = x[g*512+b, k*128+p]          (bf16, shared by cores)
      wP[p, k, o]     = (W*M)[c*512+o, k*128+p]      (bf16, per core)
      bT[p, j]        = bias[c*512 + j*128 + p]      (f32, per core)
  - Device: per batch group g (512 cols), accumulate y over 32 k-tiles in
    fp32 PSUM (4 PSUM banks = 4 out-feature chunks of 128), add bias on
    evacuation, store y^T shard as bf16.  The weight DMA is chunked so the
    matmul stream starts as soon as the first chunks land (~13us: framework
    preamble + DMA ramp); next group's x is prefetched one group ahead.
    The last group runs j-outer/k-inner so only the final 128-row
    evacuation remains in the tail.
  - Host concatenates the 8 shards, inverts the permutation, casts to f32.
"""

import os

import numpy as np
import ml_dtypes

BATCH = 4096
IN_F = 4096
OUT_F = 4096
N_CORES = 8
O_SHARD = OUT_F // N_CORES  # 512
P = 128                     # SBUF partitions
BG = 512                    # batch columns per PSUM accumulation group
WARMUP = 13                 # scratch matmuls to lift the PE HAM clock gate:
                            # sized so the (cold-rate) warmup chain ends
                            # right as the first x/w chunks land (~13.3us),
                            # so the real stream starts warm with no PE idle


def _chunks(kt):
    """k-tile DMA chunk sizes. Uniform 4-tile chunks: early DMA is
    latency-limited (~350ns/packet/engine), so the first chunk cannot
    land much before ~13us regardless of size, and smaller chunks starve
    the matmul stream during the bandwidth ramp (measured A/B)."""
    out = [4] * (kt // 4)
    rem = kt - sum(out)
    if rem:
        out.append(rem)
    return out if out else [kt]

_BF16 = ml_dtypes.bfloat16
_NC = None
LAST_RESULT = None


def _ensure_axon_hooks_stub():
    """bass_utils' axon trace path imports antenv.axon_hooks, which this
    container's antenv stub lacks. Install a minimal registry so the
    import succeeds (hook None => bass_utils skips tracing gracefully)."""
    import sys
    import types

    try:
        import antenv.axon_hooks  # noqa: F401
        return
    except ImportError:
        pass
    import antenv

    mod = types.ModuleType("antenv.axon_hooks")
    mod._HOOK = None

    def set_axon_ntff_profile_hook(h):
        mod._HOOK = h

    def get_axon_ntff_profile_hook():
        return mod._HOOK

    mod.set_axon_ntff_profile_hook = set_axon_ntff_profile_hook
    mod.get_axon_ntff_profile_hook = get_axon_ntff_profile_hook
    antenv.axon_hooks = mod
    sys.modules["antenv.axon_hooks"] = mod


def _install_real_ntff_hook():
    """Wire the ctypes NTFF profiling hook (normally registered by the
    boot middleware) so run_bass_kernel_spmd(trace=True) works."""
    _ensure_axon_hooks_stub()
    import antenv.axon_hooks as ah

    if ah.get_axon_ntff_profile_hook() is None:
        try:
            from trn_agent_boot.trn_boot import _ntff_profile_via_ctypes

            hook = _ntff_profile_via_ctypes("/opt/axon/libaxon_pjrt.so")
            if hook is not None:
                ah.set_axon_ntff_profile_hook(hook)
        except Exception:
            pass
    try:
        import concourse.bass_utils as bu

        bu.upload_artifacts = lambda tmpdir: "local://" + str(tmpdir)
    except Exception:
        pass


def build_nc(batch=BATCH, in_f=IN_F, o_shard=O_SHARD, bg=BG, warmup=WARMUP):
    import concourse.mybir as mybir
    from concourse import bacc
    from concourse.tile import TileContext

    p = P
    kt = in_f // p          # k tiles along contraction
    oc = o_shard // p       # out-feature chunks of 128
    ng = batch // bg        # batch groups
    chunks = _chunks(kt)    # k-tile chunk schedule for x and w DMA
    bf16 = mybir.dt.bfloat16
    f32 = mybir.dt.float32

    nc = bacc.Bacc()
    # flattened layouts; per-partition runs are contiguous per chunk
    xP = nc.declare_dram_parameter("xP", [p, ng * kt * bg], bf16,
                                   isOutput=False)
    wP = nc.declare_dram_parameter("wP", [p, kt * o_shard], bf16,
                                   isOutput=False)
    bT = nc.declare_dram_parameter("bT", [p, oc], f32, isOutput=False)
    yP = nc.declare_dram_parameter("yP", [p, ng * oc * bg], bf16,
                                   isOutput=True)

    with TileContext(nc) as tc:
        with tc.tile_pool(name="const", bufs=1) as cpool, \
             tc.tile_pool(name="xin", bufs=3) as xpool, \
             tc.tile_pool(name="acc", bufs=8, space="PSUM") as ppool, \
             tc.tile_pool(name="out", bufs=4) as opool:

            # scratch matmuls keep the PE busy from the end of the
            # framework preamble so the HAM clock gate opens (1.2 ->
            # 2.4 GHz) before the first real matmul's operands land
            if warmup:
                wu_w = cpool.tile([p, p], bf16)
                wu_x = cpool.tile([p, bg], bf16)
                wu_ps = ppool.tile([p, bg], f32, tag="ps", name="wu")
                nc.vector.memset(wu_w, 0.0)
                nc.vector.memset(wu_x, 0.0)
                for i in range(warmup):
                    nc.tensor.matmul(wu_ps, wu_w, wu_x,
                                     start=(i == 0),
                                     stop=(i == warmup - 1))

            # masked weights resident in SBUF; w chunks trigger on the
            # sync engine while x chunks trigger on gpsimd so the two
            # streams start in parallel (each DMA trigger costs ~0.6us
            # of engine time)
            mw = cpool.tile([p, kt * o_shard], bf16)
            xg0 = xpool.tile([p, kt * bg], bf16, tag="x")
            k0 = 0
            for ch in chunks:
                nc.sync.dma_start(
                    out=mw[:, k0 * o_shard:(k0 + ch) * o_shard],
                    in_=wP[:, k0 * o_shard:(k0 + ch) * o_shard],
                )
                nc.gpsimd.dma_start(
                    out=xg0[:, k0 * bg:(k0 + ch) * bg],
                    in_=xP[:, k0 * bg:(k0 + ch) * bg],
                )
                k0 += ch

            # bias is tiny and first needed ~40us in; trigger it last
            bias_t = cpool.tile([p, oc], f32)
            nc.sync.dma_start(out=bias_t, in_=bT[:])

            xg = xg0
            for g in range(ng):
                # prefetch next group's x one group ahead of use
                if g + 1 < ng:
                    xn = xpool.tile([p, kt * bg], bf16, tag="x")
                    base = (g + 1) * kt * bg
                    k0 = 0
                    for ch in chunks:
                        nc.gpsimd.dma_start(
                            out=xn[:, k0 * bg:(k0 + ch) * bg],
                            in_=xP[:, base + k0 * bg:base + (k0 + ch) * bg],
                        )
                        k0 += ch

                if g < ng - 1:
                    # k-outer / j-inner: x chunks are consumed as they land
                    psums = [ppool.tile([p, bg], f32, tag="ps",
                                        name=f"ps{g}_{j}")
                             for j in range(oc)]
                    for k in range(kt):
                        rhs = xg[:, k * bg:(k + 1) * bg]
                        for j in range(oc):
                            nc.tensor.matmul(
                                psums[j],
                                mw[:, k * o_shard + j * p:
                                   k * o_shard + (j + 1) * p],
                                rhs,
                                start=(k == 0),
                                stop=(k == kt - 1),
                            )
                    for j in range(oc):
                        ot = opool.tile([p, bg], bf16, tag="o")
                        nc.vector.tensor_scalar_add(
                            out=ot, in0=psums[j], scalar1=bias_t[:, j:j + 1]
                        )
                        nc.sync.dma_start(
                            out=yP[:, (g * oc + j) * bg:
                                   (g * oc + j + 1) * bg],
                            in_=ot,
                        )
                else:
                    # last group: j-outer / k-inner so earlier j's evacuate
                    # while later j's still compute -- only the final j's
                    # single full-width evacuation remains in the tail (a
                    # split-halves variant measured slower: the second DMA
                    # trigger serializes ~0.7us on the sync queue and the
                    # halved stores use 512B packets)
                    for j in range(oc):
                        ps = ppool.tile([p, bg], f32, tag="ps",
                                        name=f"ps{g}_{j}")
                        for k in range(kt):
                            nc.tensor.matmul(
                                ps,
                                mw[:, k * o_shard + j * p:
                                   k * o_shard + (j + 1) * p],
                                xg[:, k * bg:(k + 1) * bg],
                                start=(k == 0),
                                stop=(k == kt - 1),
                            )
                        base = (g * oc + j) * bg
                        ot = opool.tile([p, bg], bf16, tag="o")
                        nc.vector.tensor_scalar_add(
                            out=ot, in0=ps, scalar1=bias_t[:, j:j + 1]
                        )
                        nc.sync.dma_start(
                            out=yP[:, base:base + bg], in_=ot)
                xg = xn if g + 1 < ng else None
    nc.finalize()
    return nc


def _prep_in_maps(x, weight, bias, myFilter):
    kt = IN_F // P
    ng = BATCH // BG
    oc = O_SHARD // P
    # xP[p, g, k, b] = x[g*BG+b, k*128+p]
    xb = np.asarray(x, np.float32).astype(_BF16)
    xPb = np.ascontiguousarray(
        xb.reshape(ng, BG, kt, P).transpose(3, 0, 2, 1)
    ).reshape(P, ng * kt * BG)
    mwf = np.asarray(weight, np.float32) * np.asarray(myFilter, np.float32)
    biasf = np.asarray(bias, np.float32)
    in_maps = []
    for c in range(N_CORES):
        rows = slice(c * O_SHARD, (c + 1) * O_SHARD)
        # wP[p, k, o] = mw[c*512+o, k*128+p]
        wPb = np.ascontiguousarray(
            mwf[rows].astype(_BF16).reshape(O_SHARD, kt, P).transpose(2, 1, 0)
        ).reshape(P, kt * O_SHARD)
        bTb = np.ascontiguousarray(biasf[rows].reshape(oc, P).T)
        in_maps.append({"xP": xPb, "wP": wPb, "bT": bTb})
    return in_maps


def kernel(x, weight, bias, myFilter):
    global _NC, LAST_RESULT
    _ensure_axon_hooks_stub()
    from concourse.bass_utils import run_bass_kernel_spmd

    if _NC is None:
        _NC = build_nc()

    in_maps = _prep_in_maps(x, weight, bias, myFilter)

    kwargs = {}
    if os.environ.get("KERNEL_TRACE") == "1":
        _install_real_ntff_hook()
        kwargs["trace"] = True
        tdir = os.environ.get("KERNEL_TRACE_DIR")
        if tdir:
            kwargs["tmpdir"] = tdir

    res = run_bass_kernel_spmd(_NC, in_maps, list(range(N_CORES)), **kwargs)
    LAST_RESULT = res

    kt = IN_F // P
    ng = BATCH // BG
    oc = O_SHARD // P
    # yP[p, g, j, b] -> y[g*BG+b, c*O_SHARD + j*128 + p]
    shards = []
    for c in range(N_CORES):
        yPb = np.asarray(res.results[c]["yP"]).reshape(P, ng, oc, BG)
        shards.append(yPb.transpose(1, 3, 2, 0).reshape(BATCH, O_SHARD))
    y = np.concatenate(shards, axis=1).astype(np.float32)
    return np.ascontiguousarray(y)


# revision 22
# speedup vs baseline: 1.0102x; 1.0028x over previous
"""Locally-connected (masked linear) layer for 8 TRN2 NeuronCores.

y = x @ (W * M)^T + b
  x: [4096, 4096] f32, W/M: [4096, 4096] f32, b: [4096] f32.

Strategy (tensor-parallel over out_features):
  - Each core owns a 512-row shard of the output features.
  - The mask multiply (W * M) is exact in bf16 (M is 0/1), so it is folded
    on the host; the device only sees the pre-masked, pre-transposed weight.
  - Host uploads contraction-major, DMA-friendly layouts (>=4KB contiguous
    per partition per transfer):
      xP[p, g, k, b]  = x[g*512+b, k*128+p]          (bf16, shared by cores)
      wP[p, k, o]     = (W*M)[c*512+o, k*128+p]      (bf16, per core)
      bT[p, j]        = bias[c*512 + j*128 + p]      (f32, per core)
  - Device: per batch group g (512 cols), accumulate y over 32 k-tiles in
    fp32 PSUM (4 PSUM banks = 4 out-feature chunks of 128), add bias on
    evacuation, store y^T shard as bf16.  The weight DMA is chunked so the
    matmul stream starts as soon as the first chunks land (~13us: framework
    preamble + DMA ramp); next group's x is prefetched one group ahead.
    The last group runs j-outer/k-inner so only the final 128-row
    evacuation remains in the tail.
  - Host concatenates the 8 shards, inverts the permutation, casts to f32.
"""

import os

import numpy as np
import ml_dtypes

BATCH = 4096
IN_F = 4096
OUT_F = 4096
N_CORES = 8
O_SHARD = OUT_F // N_CORES  # 512
P = 128                     # SBUF partitions
BG = 512                    # batch columns per PSUM accumulation group
WARMUP = 32                 # scratch N=256 matmuls to lift the PE HAM clock
                            # gate: busy span (~6.8us cold-paced from ~6.9us)
                            # covers two full HAM windows, guaranteeing the
                            # un-throttle fires before the first x/w chunks
                            # land (~13.5us), so the real stream starts warm


def _chunks(kt):
    """k-tile DMA chunk sizes. Uniform 4-tile chunks: early DMA is
    latency-limited (~350ns/packet/engine), so the first chunk cannot
    land much before ~13us regardless of size, and smaller chunks starve
    the matmul stream during the bandwidth ramp (measured A/B)."""
    out = [4] * (kt // 4)
    rem = kt - sum(out)
    if rem:
        out.append(rem)
    return out if out else [kt]

_BF16 = ml_dtypes.bfloat16
_NC = None
LAST_RESULT = None


def _ensure_axon_hooks_stub():
    """bass_utils' axon trace path imports antenv.axon_hooks, which this
    container's antenv stub lacks. Install a minimal registry so the
    import succeeds (hook None => bass_utils skips tracing gracefully)."""
    import sys
    import types

    try:
        import antenv.axon_hooks  # noqa: F401
        return
    except ImportError:
        pass
    import antenv

    mod = types.ModuleType("antenv.axon_hooks")
    mod._HOOK = None

    def set_axon_ntff_profile_hook(h):
        mod._HOOK = h

    def get_axon_ntff_profile_hook():
        return mod._HOOK

    mod.set_axon_ntff_profile_hook = set_axon_ntff_profile_hook
    mod.get_axon_ntff_profile_hook = get_axon_ntff_profile_hook
    antenv.axon_hooks = mod
    sys.modules["antenv.axon_hooks"] = mod


def _install_real_ntff_hook():
    """Wire the ctypes NTFF profiling hook (normally registered by the
    boot middleware) so run_bass_kernel_spmd(trace=True) works."""
    _ensure_axon_hooks_stub()
    import antenv.axon_hooks as ah

    if ah.get_axon_ntff_profile_hook() is None:
        try:
            from trn_agent_boot.trn_boot import _ntff_profile_via_ctypes

            hook = _ntff_profile_via_ctypes("/opt/axon/libaxon_pjrt.so")
            if hook is not None:
                ah.set_axon_ntff_profile_hook(hook)
        except Exception:
            pass
    try:
        import concourse.bass_utils as bu

        bu.upload_artifacts = lambda tmpdir: "local://" + str(tmpdir)
    except Exception:
        pass


def build_nc(batch=BATCH, in_f=IN_F, o_shard=O_SHARD, bg=BG, warmup=WARMUP):
    import concourse.mybir as mybir
    from concourse import bacc
    from concourse.tile import TileContext

    p = P
    kt = in_f // p          # k tiles along contraction
    oc = o_shard // p       # out-feature chunks of 128
    ng = batch // bg        # batch groups
    chunks = _chunks(kt)    # k-tile chunk schedule for x and w DMA
    bf16 = mybir.dt.bfloat16
    f32 = mybir.dt.float32

    nc = bacc.Bacc()
    # flattened layouts; per-partition runs are contiguous per chunk
    xP = nc.declare_dram_parameter("xP", [p, ng * kt * bg], bf16,
                                   isOutput=False)
    wP = nc.declare_dram_parameter("wP", [p, kt * o_shard], bf16,
                                   isOutput=False)
    bT = nc.declare_dram_parameter("bT", [p, oc], f32, isOutput=False)
    yP = nc.declare_dram_parameter("yP", [p, ng * oc * bg], bf16,
                                   isOutput=True)

    with TileContext(nc) as tc:
        with tc.tile_pool(name="const", bufs=1) as cpool, \
             tc.tile_pool(name="xin", bufs=3) as xpool, \
             tc.tile_pool(name="acc", bufs=8, space="PSUM") as ppool, \
             tc.tile_pool(name="out", bufs=4) as opool:

            # scratch matmuls keep the PE busy from the end of the
            # framework preamble so the HAM clock gate opens (1.2 ->
            # 2.4 GHz) before the first real matmul's operands land;
            # small N=256 rhs keeps the gating memsets cheap
            if warmup:
                wu_n = min(256, bg)
                wu_w = cpool.tile([p, p], bf16)
                wu_x = cpool.tile([p, wu_n], bf16)
                wu_ps = ppool.tile([p, wu_n], f32, tag="ps", name="wu")
                nc.vector.memset(wu_x, 0.0)
                nc.vector.memset(wu_w, 0.0)
                for i in range(warmup):
                    nc.tensor.matmul(wu_ps, wu_w, wu_x,
                                     start=(i == 0),
                                     stop=(i == warmup - 1))

            # masked weights resident in SBUF; w chunks trigger on the
            # sync engine while x chunks trigger on gpsimd so the two
            # streams start in parallel (each DMA trigger costs ~0.6us
            # of engine time)
            mw = cpool.tile([p, kt * o_shard], bf16)
            xg0 = xpool.tile([p, kt * bg], bf16, tag="x")
            k0 = 0
            for ch in chunks:
                nc.sync.dma_start(
                    out=mw[:, k0 * o_shard:(k0 + ch) * o_shard],
                    in_=wP[:, k0 * o_shard:(k0 + ch) * o_shard],
                )
                nc.gpsimd.dma_start(
                    out=xg0[:, k0 * bg:(k0 + ch) * bg],
                    in_=xP[:, k0 * bg:(k0 + ch) * bg],
                )
                k0 += ch

            # bias is tiny and first needed ~40us in; trigger it last
            bias_t = cpool.tile([p, oc], f32)
            nc.sync.dma_start(out=bias_t, in_=bT[:])

            xg = xg0
            for g in range(ng):
                # prefetch next group's x one group ahead of use
                if g + 1 < ng:
                    xn = xpool.tile([p, kt * bg], bf16, tag="x")
                    base = (g + 1) * kt * bg
                    k0 = 0
                    for ch in chunks:
                        nc.gpsimd.dma_start(
                            out=xn[:, k0 * bg:(k0 + ch) * bg],
                            in_=xP[:, base + k0 * bg:base + (k0 + ch) * bg],
                        )
                        k0 += ch

                if g < ng - 1:
                    # k-outer / j-inner: x chunks are consumed as they land
                    psums = [ppool.tile([p, bg], f32, tag="ps",
                                        name=f"ps{g}_{j}")
                             for j in range(oc)]
                    for k in range(kt):
                        rhs = xg[:, k * bg:(k + 1) * bg]
                        for j in range(oc):
                            nc.tensor.matmul(
                                psums[j],
                                mw[:, k * o_shard + j * p:
                                   k * o_shard + (j + 1) * p],
                                rhs,
                                start=(k == 0),
                                stop=(k == kt - 1),
                            )
                    for j in range(oc):
                        ot = opool.tile([p, bg], bf16, tag="o")
                        nc.vector.tensor_scalar_add(
                            out=ot, in0=psums[j], scalar1=bias_t[:, j:j + 1]
                        )
                        nc.sync.dma_start(
                            out=yP[:, (g * oc + j) * bg:
                                   (g * oc + j + 1) * bg],
                            in_=ot,
                        )
                else:
                    # last group: j-outer / k-inner so earlier j's evacuate
                    # while later j's still compute -- only the final j's
                    # single full-width evacuation remains in the tail (a
                    # split-halves variant measured slower: the second DMA
                    # trigger serializes ~0.7us on the sync queue and the
                    # halved stores use 512B packets)
                    for j in range(oc):
                        ps = ppool.tile([p, bg], f32, tag="ps",
                                        name=f"ps{g}_{j}")
                        for k in range(kt):
                            nc.tensor.matmul(
                                ps,
                                mw[:, k * o_shard + j * p:
                                   k * o_shard + (j + 1) * p],
                                xg[:, k * bg:(k + 1) * bg],
                                start=(k == 0),
                                stop=(k == kt - 1),
                            )
                        base = (g * oc + j) * bg
                        ot = opool.tile([p, bg], bf16, tag="o")
                        nc.vector.tensor_scalar_add(
                            out=ot, in0=ps, scalar1=bias_t[:, j:j + 1]
                        )
                        nc.sync.dma_start(
                            out=yP[:, base:base + bg], in_=ot)
                xg = xn if g + 1 < ng else None
    nc.finalize()
    return nc


def _prep_in_maps(x, weight, bias, myFilter):
    kt = IN_F // P
    ng = BATCH // BG
    oc = O_SHARD // P
    # xP[p, g, k, b] = x[g*BG+b, k*128+p]
    xb = np.asarray(x, np.float32).astype(_BF16)
    xPb = np.ascontiguousarray(
        xb.reshape(ng, BG, kt, P).transpose(3, 0, 2, 1)
    ).reshape(P, ng * kt * BG)
    mwf = np.asarray(weight, np.float32) * np.asarray(myFilter, np.float32)
    biasf = np.asarray(bias, np.float32)
    in_maps = []
    for c in range(N_CORES):
        rows = slice(c * O_SHARD, (c + 1) * O_SHARD)
        # wP[p, k, o] = mw[c*512+o, k*128+p]
        wPb = np.ascontiguousarray(
            mwf[rows].astype(_BF16).reshape(O_SHARD, kt, P).transpose(2, 1, 0)
        ).reshape(P, kt * O_SHARD)
        bTb = np.ascontiguousarray(biasf[rows].reshape(oc, P).T)
        in_maps.append({"xP": xPb, "wP": wPb, "bT": bTb})
    return in_maps


def kernel(x, weight, bias, myFilter):
    global _NC, LAST_RESULT
    _ensure_axon_hooks_stub()
    from concourse.bass_utils import run_bass_kernel_spmd

    if _NC is None:
        _NC = build_nc()

    in_maps = _prep_in_maps(x, weight, bias, myFilter)

    kwargs = {}
    if os.environ.get("KERNEL_TRACE") == "1":
        _install_real_ntff_hook()
        kwargs["trace"] = True
        tdir = os.environ.get("KERNEL_TRACE_DIR")
        if tdir:
            kwargs["tmpdir"] = tdir

    res = run_bass_kernel_spmd(_NC, in_maps, list(range(N_CORES)), **kwargs)
    LAST_RESULT = res

    kt = IN_F // P
    ng = BATCH // BG
    oc = O_SHARD // P
    # yP[p, g, j, b] -> y[g*BG+b, c*O_SHARD + j*128 + p]
    shards = []
    for c in range(N_CORES):
        yPb = np.asarray(res.results[c]["yP"]).reshape(P, ng, oc, BG)
        shards.append(yPb.transpose(1, 3, 2, 0).reshape(BATCH, O_SHARD))
    y = np.concatenate(shards, axis=1).astype(np.float32)
    return np.ascontiguousarray(y)
